# revision 28
# baseline (speedup 1.0000x reference)
"""Trainium2 Bass kernel for CustomTradingLoss.

Computes, over B=8388608 samples with C=3 classes:
    ce      = logsumexp(pred) - pred[target]          (per sample)
    loss    = 0.85 * mean(ce * |pc|) / (mean(|pc|) + 1e-8)
            + 0.15 * mean(ce)
            + 0.1  * mean(where(aligned, -0.1, 0))
    aligned = (td > 0 & t == 2) | (td < 0 & t == 0)  == ((t-1)*td > 0)

Pure data parallel across 8 NeuronCores: core c gets samples
[c*B/8, (c+1)*B/8), laid out [128 partitions x 8192 free]. Each core
emits partial sums; the host reduces them in f64 and applies the final
formula (the three means only need global sums, so no collectives).

The on-device datapath runs in bf16 (inputs are cast host-side):
  - halves HBM traffic (the kernel is memory-bound at f32)
  - unlocks DVE 2x/4x perf modes (fp32 tensor_tensor is capped at 1x)
Targets {0,1,2} and all signs are exact in bf16; the quantization noise
on ce is ~0.4% zero-mean per sample and averages out over 8.4M samples
(measured end-to-end rel err ~1e-4 vs the f32 reference).

Engine placement notes (hardware-measured):
  - GpSimd must stay IDLE: any Pool op holds the DVE-shared SBUF port
    for its whole (slow) duration, stalling every 2-input DVE op.
  - tensor_tensor_reduce crashes this HW; sums of ce/w/al instead go
    through the otherwise-idle PE as ones-vector matmuls accumulating
    in PSUM (f32), which costs the DVE nothing.
  - bass's activation-table chooser is first-match; without forcing a
    single combined exp+ln set it reloads tables every tile.
"""

import os
import sys

import numpy as np

for _p in ("/opt/trn_rl_repo", "/opt/trn_rl_repo/concourse"):
    if os.path.isdir(_p) and _p not in sys.path:
        sys.path.insert(0, _p)

import ml_dtypes

import concourse.bacc as bacc
import concourse.mybir as mybir
import concourse.tile as tile
from concourse.bass_utils import run_bass_kernel_spmd

B = 8388608
C = 3
N_CORES = 8
N_PER_CORE = B // N_CORES  # 1048576
P = 128
F = N_PER_CORE // P  # 8192 free elements per partition
T = 2048  # tile free size

DIRECTIONAL_WEIGHT = 0.85
MAGNITUDE_WEIGHT = 0.15
TREND_WEIGHT = 0.1
EPS = 1e-8

f32 = mybir.dt.float32
bf16 = mybir.dt.bfloat16
u16 = mybir.dt.uint16
AF = mybir.ActivationFunctionType
OP = mybir.AluOpType
BF16 = ml_dtypes.bfloat16


def _force_single_act_table():
    """Make both bass and walrus use natural_log_exp_and_others (covers
    exp, ln, abs, copy, relu...) as the only activation table set, as set
    id 0 on both sides. Without this, bass's first-match set chooser
    alternates exp/ln table loads every tile (~1.3us each + a bubble).

    Two halves that must stay consistent:
      - bass picks set ids from hw_specs.get_activation_tables -> patch
        bacc's binding to a single-entry dict (id 0 = the combined set)
      - walrus validates/loads ids against act_info.json -> point
        BASS_ACT_ROOT_JSON_PATH at a filtered copy with just that set
    """
    import concourse.hw_specs as hw_specs

    name = "natural_log_exp_and_others"
    tables = hw_specs.get_activation_tables("gen3")
    if name in tables:
        bacc.get_activation_tables = lambda arch: {name: tables[name]}

    if os.environ.get("BASS_ACT_ROOT_JSON_PATH"):
        return
    import glob
    import json
    import shutil
    import tempfile

    import neuronxcc

    hits = glob.glob(
        os.path.join(os.path.dirname(neuronxcc.__file__), "pwp", "*", "act_info.json")
    )
    if not hits:
        return
    src = hits[0]
    d = json.load(open(src))
    keep = [s for s in d.get("act_func_sets", []) if s.get("name") == name]
    if not keep:
        return
    tmpdir = tempfile.mkdtemp(prefix="act_single_")
    for fn in os.listdir(os.path.dirname(src)):
        srcf = os.path.join(os.path.dirname(src), fn)
        if os.path.isfile(srcf) and fn != "act_info.json":
            try:
                os.symlink(srcf, os.path.join(tmpdir, fn))
            except OSError:
                shutil.copy(srcf, os.path.join(tmpdir, fn))
    d["act_func_sets"] = keep
    with open(os.path.join(tmpdir, "act_info.json"), "w") as f:
        json.dump(d, f)
    os.environ["BASS_ACT_ROOT_JSON_PATH"] = os.path.join(tmpdir, "act_info.json")


def _tile_sizes(f, t):
    """Short leading tiles (cheap pipeline fill), then full tiles."""
    sizes = [t // 4, t // 4, t // 2] + [t] * (f // t - 1)
    assert sum(sizes) == f
    return sizes


def build(p=P, f=F, t=T, inp_bufs=2, work_bufs=3):
    """Build + compile the per-core program. Same program on all 8 cores.

    Inputs (bf16, packed host-side):
      pred [p, f, 3]   per-sample class logits (interleaved)
      aux  [p, 3*f]    per tile k: [targets | price_changes | trend] blocks
                       of that tile's size, concatenated in tile order
    Outputs (f32): ce/w/al/ap [1, nsum] column partial sums (PE/PSUM).
    """
    _force_single_act_table()
    sizes = _tile_sizes(f, t)
    offs = [sum(sizes[:i]) for i in range(len(sizes))]
    last = len(sizes) - 1
    chunk = min(512, min(sizes))  # PE matmul column width; divides every size
    assert all(s % chunk == 0 for s in sizes)
    nsum = chunk

    nc = bacc.Bacc(
        "TRN2", target_bir_lowering=False, debug=False, num_devices=N_CORES
    )

    pred = nc.dram_tensor("pred", [p, f, C], bf16, kind="ExternalInput").ap()
    aux = nc.dram_tensor("aux", [p, 3 * f], bf16, kind="ExternalInput").ap()
    ce_out = nc.dram_tensor("ce_out", [1, nsum], f32, kind="ExternalOutput").ap()
    w_out = nc.dram_tensor("w_out", [1, nsum], f32, kind="ExternalOutput").ap()
    al_out = nc.dram_tensor("al_out", [1, nsum], f32, kind="ExternalOutput").ap()
    ap_out = nc.dram_tensor("ap_out", [1, nsum], f32, kind="ExternalOutput").ap()

    with tile.TileContext(nc) as tc:
        with (
            tc.tile_pool(name="inp", bufs=inp_bufs) as inp,
            tc.tile_pool(name="work", bufs=work_bufs) as work,
            tc.tile_pool(name="acc", bufs=1) as acc,
            tc.tile_pool(name="psum", bufs=1, space="PSUM") as psum,
        ):
            ones = acc.tile([p, 1], bf16, tag="ones")
            nc.vector.memset(ones[:], 1.0)
            ps_ce = psum.tile([1, nsum], f32, tag="ps_ce")
            ps_w = psum.tile([1, nsum], f32, tag="ps_w")
            ps_al = psum.tile([1, nsum], f32, tag="ps_al")
            ps_ap = psum.tile([1, nsum], f32, tag="ps_ap")

            def pe_sum(ps, x, k, tk):
                for j in range(tk // chunk):
                    nc.tensor.matmul(
                        ps[:],
                        ones[:],
                        x[:, j * chunk : (j + 1) * chunk],
                        start=(k == 0 and j == 0),
                        stop=(k == last and j == tk // chunk - 1),
                    )

            for k, (off, tk) in enumerate(zip(offs, sizes)):
                # ax first: the mask/trend chain only needs ax, so it can
                # start while the (3x larger) pt transfer still streams
                ax = inp.tile([p, 3, tk], bf16, tag="ax")
                axd = aux[:, 3 * off : 3 * (off + tk)].rearrange(
                    "p (c t) -> p c t", c=3
                )
                nc.sync.dma_start(out=ax[:], in_=axd[:])
                pt = inp.tile([p, tk, C], bf16, tag="pt")
                nc.sync.dma_start(out=pt[:], in_=pred[:, off : off + tk, :])
                tt = ax[:, 0, :]
                pct = ax[:, 1, :]
                tdt = ax[:, 2, :]

                # e_j = exp(pred_j), deinterleaved to unit-stride bf16 (ACT)
                e0 = work.tile([p, tk], bf16, tag="e0")
                e1 = work.tile([p, tk], bf16, tag="e1")
                e2 = work.tile([p, tk], bf16, tag="e2")
                nc.scalar.activation(e0[:], pt[:, :, 0], AF.Exp)
                nc.scalar.activation(e1[:], pt[:, :, 1], AF.Exp)
                nc.scalar.activation(e2[:], pt[:, :, 2], AF.Exp)

                # s = e0 + e1 + e2 (DVE bf16 2x); lse = ln(s) (ACT)
                s01 = work.tile([p, tk], bf16, tag="s01")
                nc.vector.tensor_add(s01[:], e0[:], e1[:])
                s = work.tile([p, tk], bf16, tag="s")
                nc.vector.tensor_add(s[:], s01[:], e2[:])
                lse = work.tile([p, tk], bf16, tag="lse")
                nc.scalar.activation(lse[:], s[:], AF.Ln)

                # masks for target selection: bf16 is_equal runs at DVE 4x;
                # the 1.0/0.0 bf16 pattern bitcasts to a valid uint16
                # predicate for copy_predicated. (GpSimd must stay idle --
                # see module docstring)
                m0 = work.tile([p, tk], bf16, tag="m0")
                nc.vector.tensor_scalar(
                    out=m0[:], in0=tt, scalar1=0.0, scalar2=None, op0=OP.is_equal
                )
                m2 = work.tile([p, tk], bf16, tag="m2")
                nc.vector.tensor_scalar(
                    out=m2[:], in0=tt, scalar1=2.0, scalar2=None, op0=OP.is_equal
                )

                # e1 <- e[target] via predicated overwrites (DVE), then ln
                nc.vector.copy_predicated(
                    out=e1[:], mask=m2[:].bitcast(u16), data=e2[:]
                )
                nc.vector.copy_predicated(
                    out=e1[:], mask=m0[:].bitcast(u16), data=e0[:]
                )
                lsel = work.tile([p, tk], bf16, tag="lsel")
                nc.scalar.activation(lsel[:], e1[:], AF.Ln)

                # ap = |pc| by clearing the bf16 sign bit (DVE 4x int op;
                # cheaper than an ACT Abs pass). Sum goes through PE.
                apb = work.tile([p, tk], u16, tag="apb")
                nc.vector.tensor_scalar(
                    out=apb[:],
                    in0=pct.bitcast(u16),
                    scalar1=0x7FFF,
                    scalar2=None,
                    op0=OP.bitwise_and,
                )
                apt = apb[:].bitcast(bf16)

                # ce = lse - lsel; w = ce * ap  (DVE 2x TT; sums on PE)
                ce = work.tile([p, tk], bf16, tag="ce")
                nc.vector.tensor_sub(ce[:], lse[:], lsel[:])
                w = work.tile([p, tk], bf16, tag="w")
                nc.vector.tensor_mul(w[:], ce[:], apt)

                # aligned = ((t-1)*td > 0)  (DVE; sum on PE)
                u = work.tile([p, tk], bf16, tag="s01")
                nc.vector.tensor_scalar(
                    out=u[:], in0=tt, scalar1=1.0, scalar2=None, op0=OP.subtract
                )
                q = work.tile([p, tk], bf16, tag="s")
                nc.vector.tensor_mul(q[:], u[:], tdt)
                al = work.tile([p, tk], bf16, tag="lse")
                nc.vector.tensor_scalar(
                    out=al[:], in0=q[:], scalar1=0.0, scalar2=None, op0=OP.is_gt
                )

                pe_sum(ps_ce, ce[:], k, tk)
                pe_sum(ps_w, w[:], k, tk)
                pe_sum(ps_al, al[:], k, tk)
                pe_sum(ps_ap, apt, k, tk)

            sums = acc.tile([1, 4, nsum], f32, tag="sums")
            nc.vector.tensor_copy(out=sums[:, 0, :], in_=ps_ce[:])
            nc.vector.tensor_copy(out=sums[:, 1, :], in_=ps_w[:])
            nc.vector.tensor_copy(out=sums[:, 2, :], in_=ps_al[:])
            nc.vector.tensor_copy(out=sums[:, 3, :], in_=ps_ap[:])
            nc.sync.dma_start(out=ce_out[:], in_=sums[:, 0, :])
            nc.sync.dma_start(out=w_out[:], in_=sums[:, 1, :])
            nc.sync.dma_start(out=al_out[:], in_=sums[:, 2, :])
            nc.sync.dma_start(out=ap_out[:], in_=sums[:, 3, :])

    nc.compile()
    return nc


_NC = None


def _get_nc():
    global _NC
    if _NC is None:
        _NC = build()
    return _NC


def make_in_maps(predictions, targets, price_changes, trend_direction, p=P, t=T):
    """Shard across cores and pack into the kernel's bf16 input layout."""
    predictions = np.asarray(predictions)
    targets = np.asarray(targets)
    price_changes = np.asarray(price_changes)
    trend_direction = np.asarray(trend_direction)

    n = predictions.shape[0]
    n_per_core = n // N_CORES
    f = n_per_core // p
    sizes = _tile_sizes(f, t)
    offs = [sum(sizes[:i]) for i in range(len(sizes))]

    pred_bf = predictions.astype(BF16)
    tgt_bf = targets.astype(BF16)
    pc_bf = price_changes.astype(BF16)
    td_bf = trend_direction.astype(BF16)

    in_maps = []
    for c in range(N_CORES):
        sl = slice(c * n_per_core, (c + 1) * n_per_core)
        tg = tgt_bf[sl].reshape(p, f)
        pc2 = pc_bf[sl].reshape(p, f)
        td2 = td_bf[sl].reshape(p, f)
        blocks = []
        for off, tk in zip(offs, sizes):
            blocks.append(tg[:, off : off + tk])
            blocks.append(pc2[:, off : off + tk])
            blocks.append(td2[:, off : off + tk])
        auxv = np.concatenate(blocks, axis=1)  # [p, 3*f]
        in_maps.append(
            {
                "pred": np.ascontiguousarray(pred_bf[sl]).reshape(p, f, C),
                "aux": np.ascontiguousarray(auxv),
            }
        )
    return in_maps


def combine(results):
    """Host-side reduction of per-core partial sums -> final scalar loss."""
    s_ce = s_w = s_ap = s_al = 0.0
    for r in results:
        s_ce += float(r["ce_out"].astype(np.float64).sum())
        s_w += float(r["w_out"].astype(np.float64).sum())
        s_ap += float(r["ap_out"].astype(np.float64).sum())
        s_al += float(r["al_out"].astype(np.float64).sum())

    mean_ap = s_ap / B
    weighted_ce_mean = (s_w / B) / (mean_ap + EPS)
    ce_mean = s_ce / B
    trend_mean = -0.1 * s_al / B
    loss = (
        DIRECTIONAL_WEIGHT * weighted_ce_mean
        + MAGNITUDE_WEIGHT * ce_mean
        + TREND_WEIGHT * trend_mean
    )
    return np.float32(loss)


def kernel(predictions, targets, price_changes, trend_direction):
    nc = _get_nc()
    in_maps = make_in_maps(predictions, targets, price_changes, trend_direction)
    last_err = None
    for _attempt in range(3):
        try:
            res = run_bass_kernel_spmd(nc, in_maps, core_ids=list(range(N_CORES)))
            return combine(res.results)
        except Exception as e:  # rare transient NRT_EXEC_UNIT_UNRECOVERABLE
            last_err = e
    raise last_err


# revision 29
# speedup vs baseline: 1.0262x; 1.0262x over previous
"""Trainium2 Bass kernel for CustomTradingLoss.

Computes, over B=8388608 samples with C=3 classes:
    ce      = logsumexp(pred) - pred[target]          (per sample)
    loss    = 0.85 * mean(ce * |pc|) / (mean(|pc|) + 1e-8)
            + 0.15 * mean(ce)
            + 0.1  * mean(where(aligned, -0.1, 0))
    aligned = (td > 0 & t == 2) | (td < 0 & t == 0)  == ((t-1)*td > 0)

Pure data parallel across 8 NeuronCores: core c gets samples
[c*B/8, (c+1)*B/8), laid out [128 partitions x 8192 free]. Each core
emits partial sums; the host reduces them in f64 and applies the final
formula (the three means only need global sums, so no collectives).

The on-device datapath runs in bf16 (inputs are cast host-side):
  - halves HBM traffic (the kernel is memory-bound at f32)
  - unlocks DVE 2x/4x perf modes (fp32 tensor_tensor is capped at 1x)
Targets {0,1,2} and all signs are exact in bf16; the quantization noise
on ce is ~0.4% zero-mean per sample and averages out over 8.4M samples
(measured end-to-end rel err ~1e-4 vs the f32 reference).

Engine placement notes (hardware-measured):
  - GpSimd must stay IDLE: any Pool op holds the DVE-shared SBUF port
    for its whole (slow) duration, stalling every 2-input DVE op.
  - tensor_tensor_reduce crashes this HW; sums of ce/w/al instead go
    through the otherwise-idle PE as ones-vector matmuls accumulating
    in PSUM (f32), which costs the DVE nothing.
  - bass's activation-table chooser is first-match; without forcing a
    single combined exp+ln set it reloads tables every tile.
"""

import os
import sys

import numpy as np

for _p in ("/opt/trn_rl_repo", "/opt/trn_rl_repo/concourse"):
    if os.path.isdir(_p) and _p not in sys.path:
        sys.path.insert(0, _p)

import ml_dtypes

import concourse.bacc as bacc
import concourse.mybir as mybir
import concourse.tile as tile
from concourse.bass_utils import run_bass_kernel_spmd

B = 8388608
C = 3
N_CORES = 8
N_PER_CORE = B // N_CORES  # 1048576
P = 128
F = N_PER_CORE // P  # 8192 free elements per partition
T = 2048  # tile free size

DIRECTIONAL_WEIGHT = 0.85
MAGNITUDE_WEIGHT = 0.15
TREND_WEIGHT = 0.1
EPS = 1e-8

f32 = mybir.dt.float32
bf16 = mybir.dt.bfloat16
u16 = mybir.dt.uint16
AF = mybir.ActivationFunctionType
OP = mybir.AluOpType
BF16 = ml_dtypes.bfloat16


def _force_single_act_table():
    """Make both bass and walrus use natural_log_exp_and_others (covers
    exp, ln, abs, copy, relu...) as the only activation table set, as set
    id 0 on both sides. Without this, bass's first-match set chooser
    alternates exp/ln table loads every tile (~1.3us each + a bubble).

    Two halves that must stay consistent:
      - bass picks set ids from hw_specs.get_activation_tables -> patch
        bacc's binding to a single-entry dict (id 0 = the combined set)
      - walrus validates/loads ids against act_info.json -> point
        BASS_ACT_ROOT_JSON_PATH at a filtered copy with just that set
    """
    import concourse.hw_specs as hw_specs

    name = "natural_log_exp_and_others"
    tables = hw_specs.get_activation_tables("gen3")
    if name in tables:
        bacc.get_activation_tables = lambda arch: {name: tables[name]}

    if os.environ.get("BASS_ACT_ROOT_JSON_PATH"):
        return
    import glob
    import json
    import shutil
    import tempfile

    import neuronxcc

    hits = glob.glob(
        os.path.join(os.path.dirname(neuronxcc.__file__), "pwp", "*", "act_info.json")
    )
    if not hits:
        return
    src = hits[0]
    d = json.load(open(src))
    keep = [s for s in d.get("act_func_sets", []) if s.get("name") == name]
    if not keep:
        return
    tmpdir = tempfile.mkdtemp(prefix="act_single_")
    for fn in os.listdir(os.path.dirname(src)):
        srcf = os.path.join(os.path.dirname(src), fn)
        if os.path.isfile(srcf) and fn != "act_info.json":
            try:
                os.symlink(srcf, os.path.join(tmpdir, fn))
            except OSError:
                shutil.copy(srcf, os.path.join(tmpdir, fn))
    d["act_func_sets"] = keep
    with open(os.path.join(tmpdir, "act_info.json"), "w") as f:
        json.dump(d, f)
    os.environ["BASS_ACT_ROOT_JSON_PATH"] = os.path.join(tmpdir, "act_info.json")


def _tile_sizes(f, t):
    """Short leading tiles (cheap pipeline fill), then full tiles."""
    sizes = [t // 4, t // 4, t // 2] + [t] * (f // t - 1)
    assert sum(sizes) == f
    return sizes


def build(p=P, f=F, t=T, inp_bufs=3, work_bufs=2):
    """Build + compile the per-core program. Same program on all 8 cores.

    Inputs (bf16, packed host-side):
      pred [p, f, 3]   per-sample class logits (interleaved)
      aux  [p, 3*f]    per tile k: [targets | price_changes | trend] blocks
                       of that tile's size, concatenated in tile order
    Outputs (f32): ce/w/al/ap [1, nsum] column partial sums (PE/PSUM).
    """
    _force_single_act_table()
    sizes = _tile_sizes(f, t)
    offs = [sum(sizes[:i]) for i in range(len(sizes))]
    last = len(sizes) - 1
    chunk = min(512, min(sizes))  # PE matmul column width; divides every size
    assert all(s % chunk == 0 for s in sizes)
    nsum = chunk

    nc = bacc.Bacc(
        "TRN2", target_bir_lowering=False, debug=False, num_devices=N_CORES
    )

    pred = nc.dram_tensor("pred", [p, f, C], bf16, kind="ExternalInput").ap()
    aux = nc.dram_tensor("aux", [p, 3 * f], bf16, kind="ExternalInput").ap()
    ce_out = nc.dram_tensor("ce_out", [1, nsum], f32, kind="ExternalOutput").ap()
    w_out = nc.dram_tensor("w_out", [1, nsum], f32, kind="ExternalOutput").ap()
    al_out = nc.dram_tensor("al_out", [1, nsum], f32, kind="ExternalOutput").ap()
    ap_out = nc.dram_tensor("ap_out", [1, nsum], f32, kind="ExternalOutput").ap()

    with tile.TileContext(nc) as tc:
        with (
            tc.tile_pool(name="inp", bufs=inp_bufs) as inp,
            tc.tile_pool(name="work", bufs=work_bufs) as work,
            tc.tile_pool(name="acc", bufs=1) as acc,
            tc.tile_pool(name="psum", bufs=1, space="PSUM") as psum,
        ):
            ones = acc.tile([p, 1], bf16, tag="ones")
            nc.vector.memset(ones[:], 1.0)
            ps_ce = psum.tile([1, nsum], f32, tag="ps_ce")
            ps_w = psum.tile([1, nsum], f32, tag="ps_w")
            ps_al = psum.tile([1, nsum], f32, tag="ps_al")
            ps_ap = psum.tile([1, nsum], f32, tag="ps_ap")

            def pe_sum(ps, x, k, tk):
                for j in range(tk // chunk):
                    nc.tensor.matmul(
                        ps[:],
                        ones[:],
                        x[:, j * chunk : (j + 1) * chunk],
                        start=(k == 0 and j == 0),
                        stop=(k == last and j == tk // chunk - 1),
                    )

            for k, (off, tk) in enumerate(zip(offs, sizes)):
                # ax first: the mask/trend chain only needs ax, so it can
                # start while the (3x larger) pt transfer still streams
                ax = inp.tile([p, 3, tk], bf16, tag="ax")
                axd = aux[:, 3 * off : 3 * (off + tk)].rearrange(
                    "p (c t) -> p c t", c=3
                )
                nc.sync.dma_start(out=ax[:], in_=axd[:])
                pt = inp.tile([p, tk, C], bf16, tag="pt")
                nc.sync.dma_start(out=pt[:], in_=pred[:, off : off + tk, :])
                tt = ax[:, 0, :]
                pct = ax[:, 1, :]
                tdt = ax[:, 2, :]

                # e_j = exp(pred_j), deinterleaved to unit-stride bf16 (ACT)
                e0 = work.tile([p, tk], bf16, tag="e0")
                e1 = work.tile([p, tk], bf16, tag="e1")
                e2 = work.tile([p, tk], bf16, tag="e2")
                nc.scalar.activation(e0[:], pt[:, :, 0], AF.Exp)
                nc.scalar.activation(e1[:], pt[:, :, 1], AF.Exp)
                nc.scalar.activation(e2[:], pt[:, :, 2], AF.Exp)

                # s = e0 + e1 + e2 (DVE bf16 2x); lse = ln(s) (ACT)
                s01 = work.tile([p, tk], bf16, tag="s01")
                nc.vector.tensor_add(s01[:], e0[:], e1[:])
                s = work.tile([p, tk], bf16, tag="s")
                nc.vector.tensor_add(s[:], s01[:], e2[:])
                lse = work.tile([p, tk], bf16, tag="lse")
                nc.scalar.activation(lse[:], s[:], AF.Ln)

                # masks for target selection: bf16 is_equal runs at DVE 4x;
                # the 1.0/0.0 bf16 pattern bitcasts to a valid uint16
                # predicate for copy_predicated. (GpSimd must stay idle --
                # see module docstring)
                m0 = work.tile([p, tk], bf16, tag="m0")
                nc.vector.tensor_scalar(
                    out=m0[:], in0=tt, scalar1=0.0, scalar2=None, op0=OP.is_equal
                )
                m2 = work.tile([p, tk], bf16, tag="m2")
                nc.vector.tensor_scalar(
                    out=m2[:], in0=tt, scalar1=2.0, scalar2=None, op0=OP.is_equal
                )

                # e1 <- e[target] via predicated overwrites (DVE), then ln
                nc.vector.copy_predicated(
                    out=e1[:], mask=m2[:].bitcast(u16), data=e2[:]
                )
                nc.vector.copy_predicated(
                    out=e1[:], mask=m0[:].bitcast(u16), data=e0[:]
                )
                lsel = work.tile([p, tk], bf16, tag="lsel")
                nc.scalar.activation(lsel[:], e1[:], AF.Ln)

                # ap = |pc| by clearing the bf16 sign bit (DVE 4x int op;
                # cheaper than an ACT Abs pass). Sum goes through PE.
                apb = work.tile([p, tk], u16, tag="apb")
                nc.vector.tensor_scalar(
                    out=apb[:],
                    in0=pct.bitcast(u16),
                    scalar1=0x7FFF,
                    scalar2=None,
                    op0=OP.bitwise_and,
                )
                apt = apb[:].bitcast(bf16)

                # ce = lse - lsel; w = ce * ap  (DVE 2x TT; sums on PE)
                ce = work.tile([p, tk], bf16, tag="ce")
                nc.vector.tensor_sub(ce[:], lse[:], lsel[:])
                w = work.tile([p, tk], bf16, tag="w")
                nc.vector.tensor_mul(w[:], ce[:], apt)

                # aligned = ((t-1)*td > 0)  (DVE; sum on PE)
                u = work.tile([p, tk], bf16, tag="u")
                nc.vector.tensor_scalar(
                    out=u[:], in0=tt, scalar1=1.0, scalar2=None, op0=OP.subtract
                )
                q = work.tile([p, tk], bf16, tag="q")
                nc.vector.tensor_mul(q[:], u[:], tdt)
                al = work.tile([p, tk], bf16, tag="al")
                nc.vector.tensor_scalar(
                    out=al[:], in0=q[:], scalar1=0.0, scalar2=None, op0=OP.is_gt
                )

                pe_sum(ps_ce, ce[:], k, tk)
                pe_sum(ps_w, w[:], k, tk)
                pe_sum(ps_al, al[:], k, tk)
                pe_sum(ps_ap, apt, k, tk)

            sums = acc.tile([1, 4, nsum], f32, tag="sums")
            nc.vector.tensor_copy(out=sums[:, 0, :], in_=ps_ce[:])
            nc.vector.tensor_copy(out=sums[:, 1, :], in_=ps_w[:])
            nc.vector.tensor_copy(out=sums[:, 2, :], in_=ps_al[:])
            nc.vector.tensor_copy(out=sums[:, 3, :], in_=ps_ap[:])
            nc.sync.dma_start(out=ce_out[:], in_=sums[:, 0, :])
            nc.sync.dma_start(out=w_out[:], in_=sums[:, 1, :])
            nc.sync.dma_start(out=al_out[:], in_=sums[:, 2, :])
            nc.sync.dma_start(out=ap_out[:], in_=sums[:, 3, :])

    nc.compile()
    return nc


_NC = None


def _get_nc():
    global _NC
    if _NC is None:
        _NC = build()
    return _NC


def make_in_maps(predictions, targets, price_changes, trend_direction, p=P, t=T):
    """Shard across cores and pack into the kernel's bf16 input layout."""
    predictions = np.asarray(predictions)
    targets = np.asarray(targets)
    price_changes = np.asarray(price_changes)
    trend_direction = np.asarray(trend_direction)

    n = predictions.shape[0]
    n_per_core = n // N_CORES
    f = n_per_core // p
    sizes = _tile_sizes(f, t)
    offs = [sum(sizes[:i]) for i in range(len(sizes))]

    pred_bf = predictions.astype(BF16)
    tgt_bf = targets.astype(BF16)
    pc_bf = price_changes.astype(BF16)
    td_bf = trend_direction.astype(BF16)

    in_maps = []
    for c in range(N_CORES):
        sl = slice(c * n_per_core, (c + 1) * n_per_core)
        tg = tgt_bf[sl].reshape(p, f)
        pc2 = pc_bf[sl].reshape(p, f)
        td2 = td_bf[sl].reshape(p, f)
        blocks = []
        for off, tk in zip(offs, sizes):
            blocks.append(tg[:, off : off + tk])
            blocks.append(pc2[:, off : off + tk])
            blocks.append(td2[:, off : off + tk])
        auxv = np.concatenate(blocks, axis=1)  # [p, 3*f]
        in_maps.append(
            {
                "pred": np.ascontiguousarray(pred_bf[sl]).reshape(p, f, C),
                "aux": np.ascontiguousarray(auxv),
            }
        )
    return in_maps


def combine(results):
    """Host-side reduction of per-core partial sums -> final scalar loss."""
    s_ce = s_w = s_ap = s_al = 0.0
    for r in results:
        s_ce += float(r["ce_out"].astype(np.float64).sum())
        s_w += float(r["w_out"].astype(np.float64).sum())
        s_ap += float(r["ap_out"].astype(np.float64).sum())
        s_al += float(r["al_out"].astype(np.float64).sum())

    mean_ap = s_ap / B
    weighted_ce_mean = (s_w / B) / (mean_ap + EPS)
    ce_mean = s_ce / B
    trend_mean = -0.1 * s_al / B
    loss = (
        DIRECTIONAL_WEIGHT * weighted_ce_mean
        + MAGNITUDE_WEIGHT * ce_mean
        + TREND_WEIGHT * trend_mean
    )
    return np.float32(loss)


def kernel(predictions, targets, price_changes, trend_direction):
    nc = _get_nc()
    in_maps = make_in_maps(predictions, targets, price_changes, trend_direction)
    last_err = None
    for _attempt in range(3):
        try:
            res = run_bass_kernel_spmd(nc, in_maps, core_ids=list(range(N_CORES)))
            return combine(res.results)
        except Exception as e:  # rare transient NRT_EXEC_UNIT_UNRECOVERABLE
            last_err = e
    raise last_err


# revision 31
# speedup vs baseline: 1.0308x; 1.0045x over previous
"""Trainium2 Bass kernel for CustomTradingLoss.

Computes, over B=8388608 samples with C=3 classes:
    ce      = logsumexp(pred) - pred[target]          (per sample)
    loss    = 0.85 * mean(ce * |pc|) / (mean(|pc|) + 1e-8)
            + 0.15 * mean(ce)
            + 0.1  * mean(where(aligned, -0.1, 0))
    aligned = (td > 0 & t == 2) | (td < 0 & t == 0)  == ((t-1)*td > 0)

Pure data parallel across 8 NeuronCores: core c gets samples
[c*B/8, (c+1)*B/8), laid out [128 partitions x 8192 free]. Each core
emits partial sums; the host reduces them in f64 and applies the final
formula (the three means only need global sums, so no collectives).

The on-device datapath runs in bf16 (inputs are cast host-side):
  - halves HBM traffic (the kernel is memory-bound at f32)
  - unlocks DVE 2x/4x perf modes (fp32 tensor_tensor is capped at 1x)
Targets {0,1,2} and all signs are exact in bf16; the quantization noise
on ce is ~0.4% zero-mean per sample and averages out over 8.4M samples
(measured end-to-end rel err ~1e-4 vs the f32 reference).

Engine placement notes (hardware-measured):
  - GpSimd must stay IDLE: any Pool op holds the DVE-shared SBUF port
    for its whole (slow) duration, stalling every 2-input DVE op.
  - tensor_tensor_reduce crashes this HW; sums of ce/w/al instead go
    through the otherwise-idle PE as ones-vector matmuls accumulating
    in PSUM (f32), which costs the DVE nothing.
  - bass's activation-table chooser is first-match; without forcing a
    single combined exp+ln set it reloads tables every tile.
"""

import os
import sys

import numpy as np

for _p in ("/opt/trn_rl_repo", "/opt/trn_rl_repo/concourse"):
    if os.path.isdir(_p) and _p not in sys.path:
        sys.path.insert(0, _p)

import ml_dtypes

import concourse.bacc as bacc
import concourse.mybir as mybir
import concourse.tile as tile
from concourse.bass_utils import run_bass_kernel_spmd

B = 8388608
C = 3
N_CORES = 8
N_PER_CORE = B // N_CORES  # 1048576
P = 128
F = N_PER_CORE // P  # 8192 free elements per partition
T = 2048  # tile free size

DIRECTIONAL_WEIGHT = 0.85
MAGNITUDE_WEIGHT = 0.15
TREND_WEIGHT = 0.1
EPS = 1e-8

f32 = mybir.dt.float32
bf16 = mybir.dt.bfloat16
u16 = mybir.dt.uint16
AF = mybir.ActivationFunctionType
OP = mybir.AluOpType
BF16 = ml_dtypes.bfloat16


def _force_single_act_table():
    """Make both bass and walrus use natural_log_exp_and_others (covers
    exp, ln, abs, copy, relu...) as the only activation table set, as set
    id 0 on both sides. Without this, bass's first-match set chooser
    alternates exp/ln table loads every tile (~1.3us each + a bubble).

    Two halves that must stay consistent:
      - bass picks set ids from hw_specs.get_activation_tables -> patch
        bacc's binding to a single-entry dict (id 0 = the combined set)
      - walrus validates/loads ids against act_info.json -> point
        BASS_ACT_ROOT_JSON_PATH at a filtered copy with just that set
    """
    import concourse.hw_specs as hw_specs

    name = "natural_log_exp_and_others"
    tables = hw_specs.get_activation_tables("gen3")
    if name in tables:
        bacc.get_activation_tables = lambda arch: {name: tables[name]}

    if os.environ.get("BASS_ACT_ROOT_JSON_PATH"):
        return
    import glob
    import json
    import shutil
    import tempfile

    import neuronxcc

    hits = glob.glob(
        os.path.join(os.path.dirname(neuronxcc.__file__), "pwp", "*", "act_info.json")
    )
    if not hits:
        return
    src = hits[0]
    d = json.load(open(src))
    keep = [s for s in d.get("act_func_sets", []) if s.get("name") == name]
    if not keep:
        return
    tmpdir = tempfile.mkdtemp(prefix="act_single_")
    for fn in os.listdir(os.path.dirname(src)):
        srcf = os.path.join(os.path.dirname(src), fn)
        if os.path.isfile(srcf) and fn != "act_info.json":
            try:
                os.symlink(srcf, os.path.join(tmpdir, fn))
            except OSError:
                shutil.copy(srcf, os.path.join(tmpdir, fn))
    d["act_func_sets"] = keep
    with open(os.path.join(tmpdir, "act_info.json"), "w") as f:
        json.dump(d, f)
    os.environ["BASS_ACT_ROOT_JSON_PATH"] = os.path.join(tmpdir, "act_info.json")


def _tile_sizes(f, t):
    """Short leading tiles (cheap pipeline fill), then full tiles."""
    sizes = [t // 4, t // 4, t // 2] + [t] * (f // t - 1)
    assert sum(sizes) == f
    return sizes


def build(p=P, f=F, t=T, inp_bufs=3, work_bufs=2):
    """Build + compile the per-core program. Same program on all 8 cores.

    Inputs (bf16, packed host-side):
      pred [p, f, 3]   per-sample class logits (interleaved)
      aux  [p, 3*f]    per tile k: [targets | price_changes | trend] blocks
                       of that tile's size, concatenated in tile order
    Outputs (f32): ce/w/al/ap [1, nsum] column partial sums (PE/PSUM).
    """
    _force_single_act_table()
    sizes = _tile_sizes(f, t)
    offs = [sum(sizes[:i]) for i in range(len(sizes))]
    last = len(sizes) - 1
    chunk = min(512, min(sizes))  # PE matmul column width; divides every size
    assert all(s % chunk == 0 for s in sizes)
    nsum = chunk

    nc = bacc.Bacc(
        "TRN2", target_bir_lowering=False, debug=False, num_devices=N_CORES
    )

    pred = nc.dram_tensor("pred", [p, f, C], bf16, kind="ExternalInput").ap()
    aux = nc.dram_tensor("aux", [p, 3 * f], bf16, kind="ExternalInput").ap()
    ce_out = nc.dram_tensor("ce_out", [1, nsum], f32, kind="ExternalOutput").ap()
    w_out = nc.dram_tensor("w_out", [1, nsum], f32, kind="ExternalOutput").ap()
    al_out = nc.dram_tensor("al_out", [1, nsum], f32, kind="ExternalOutput").ap()
    ap_out = nc.dram_tensor("ap_out", [1, nsum], f32, kind="ExternalOutput").ap()

    with tile.TileContext(nc) as tc:
        with (
            tc.tile_pool(name="inp", bufs=inp_bufs) as inp,
            tc.tile_pool(name="work", bufs=work_bufs) as work,
            tc.tile_pool(name="acc", bufs=1) as acc,
            tc.tile_pool(name="psum", bufs=1, space="PSUM") as psum,
        ):
            ones = acc.tile([p, 1], bf16, tag="ones")
            nc.vector.memset(ones[:], 1.0)
            neg1 = acc.tile([p, 1], f32, tag="neg1")
            nc.vector.memset(neg1[:], -1.0)
            ps_ce = psum.tile([1, nsum], f32, tag="ps_ce")
            ps_w = psum.tile([1, nsum], f32, tag="ps_w")
            ps_al = psum.tile([1, nsum], f32, tag="ps_al")
            ps_ap = psum.tile([1, nsum], f32, tag="ps_ap")

            def pe_sum(ps, x, k, tk):
                for j in range(tk // chunk):
                    nc.tensor.matmul(
                        ps[:],
                        ones[:],
                        x[:, j * chunk : (j + 1) * chunk],
                        start=(k == 0 and j == 0),
                        stop=(k == last and j == tk // chunk - 1),
                    )

            for k, (off, tk) in enumerate(zip(offs, sizes)):
                # ax first: the mask/trend chain only needs ax, so it can
                # start while the (3x larger) pt transfer still streams
                ax = inp.tile([p, 3, tk], bf16, tag="ax")
                axd = aux[:, 3 * off : 3 * (off + tk)].rearrange(
                    "p (c t) -> p c t", c=3
                )
                nc.sync.dma_start(out=ax[:], in_=axd[:])
                pt = inp.tile([p, tk, C], bf16, tag="pt")
                nc.sync.dma_start(out=pt[:], in_=pred[:, off : off + tk, :])
                tt = ax[:, 0, :]
                pct = ax[:, 1, :]
                tdt = ax[:, 2, :]

                # e_j = exp(pred_j), deinterleaved to unit-stride bf16 (ACT)
                e0 = work.tile([p, tk], bf16, tag="e0")
                e1 = work.tile([p, tk], bf16, tag="e1")
                e2 = work.tile([p, tk], bf16, tag="e2")
                nc.scalar.activation(e0[:], pt[:, :, 0], AF.Exp)
                nc.scalar.activation(e1[:], pt[:, :, 1], AF.Exp)
                nc.scalar.activation(e2[:], pt[:, :, 2], AF.Exp)

                # s = e0 + e1 + e2 (DVE bf16 2x); lse = ln(s) (ACT)
                s01 = work.tile([p, tk], bf16, tag="s01")
                nc.vector.tensor_add(s01[:], e0[:], e1[:])
                s = work.tile([p, tk], bf16, tag="s")
                nc.vector.tensor_add(s[:], s01[:], e2[:])
                lse = work.tile([p, tk], bf16, tag="lse")
                nc.scalar.activation(lse[:], s[:], AF.Ln)

                # masks for target selection: bf16 is_equal runs at DVE 4x;
                # the 1.0/0.0 bf16 pattern bitcasts to a valid uint16
                # predicate for copy_predicated. (GpSimd must stay idle --
                # see module docstring)
                m0 = work.tile([p, tk], bf16, tag="m0")
                nc.vector.tensor_scalar(
                    out=m0[:], in0=tt, scalar1=0.0, scalar2=None, op0=OP.is_equal
                )
                m2 = work.tile([p, tk], bf16, tag="m2")
                nc.scalar.activation(
                    m2[:], tt, AF.Relu, bias=neg1[:], scale=1.0
                )

                # e1 <- e[target] via predicated overwrites (DVE), then ln
                nc.vector.copy_predicated(
                    out=e1[:], mask=m2[:].bitcast(u16), data=e2[:]
                )
                nc.vector.copy_predicated(
                    out=e1[:], mask=m0[:].bitcast(u16), data=e0[:]
                )
                lsel = work.tile([p, tk], bf16, tag="lsel")
                nc.scalar.activation(lsel[:], e1[:], AF.Ln)

                # ap = |pc| by clearing the bf16 sign bit (DVE 4x int op;
                # cheaper than an ACT Abs pass). Sum goes through PE.
                apb = work.tile([p, tk], u16, tag="apb")
                nc.vector.tensor_scalar(
                    out=apb[:],
                    in0=pct.bitcast(u16),
                    scalar1=0x7FFF,
                    scalar2=None,
                    op0=OP.bitwise_and,
                )
                apt = apb[:].bitcast(bf16)

                # ce = lse - lsel; w = ce * ap  (DVE 2x TT; sums on PE)
                ce = work.tile([p, tk], bf16, tag="ce")
                nc.vector.tensor_sub(ce[:], lse[:], lsel[:])
                w = work.tile([p, tk], bf16, tag="w")
                nc.vector.tensor_mul(w[:], ce[:], apt)

                # aligned = ((t-1)*td > 0)  (DVE; sum on PE)
                u = work.tile([p, tk], bf16, tag="u")
                nc.vector.tensor_scalar(
                    out=u[:], in0=tt, scalar1=1.0, scalar2=None, op0=OP.subtract
                )
                q = work.tile([p, tk], bf16, tag="q")
                nc.vector.tensor_mul(q[:], u[:], tdt)
                al = work.tile([p, tk], bf16, tag="al")
                nc.vector.tensor_scalar(
                    out=al[:], in0=q[:], scalar1=0.0, scalar2=None, op0=OP.is_gt
                )

                pe_sum(ps_ce, ce[:], k, tk)
                pe_sum(ps_w, w[:], k, tk)
                pe_sum(ps_al, al[:], k, tk)
                pe_sum(ps_ap, apt, k, tk)

            sums = acc.tile([1, 4, nsum], f32, tag="sums")
            nc.scalar.copy(out=sums[:, 0, :], in_=ps_ce[:])
            nc.scalar.copy(out=sums[:, 1, :], in_=ps_w[:])
            nc.scalar.copy(out=sums[:, 2, :], in_=ps_al[:])
            nc.scalar.copy(out=sums[:, 3, :], in_=ps_ap[:])
            nc.sync.dma_start(out=ce_out[:], in_=sums[:, 0, :])
            nc.sync.dma_start(out=w_out[:], in_=sums[:, 1, :])
            nc.sync.dma_start(out=al_out[:], in_=sums[:, 2, :])
            nc.sync.dma_start(out=ap_out[:], in_=sums[:, 3, :])

    nc.compile()
    return nc


_NC = None


def _get_nc():
    global _NC
    if _NC is None:
        _NC = build()
    return _NC


def make_in_maps(predictions, targets, price_changes, trend_direction, p=P, t=T):
    """Shard across cores and pack into the kernel's bf16 input layout."""
    predictions = np.asarray(predictions)
    targets = np.asarray(targets)
    price_changes = np.asarray(price_changes)
    trend_direction = np.asarray(trend_direction)

    n = predictions.shape[0]
    n_per_core = n // N_CORES
    f = n_per_core // p
    sizes = _tile_sizes(f, t)
    offs = [sum(sizes[:i]) for i in range(len(sizes))]

    pred_bf = predictions.astype(BF16)
    tgt_bf = targets.astype(BF16)
    pc_bf = price_changes.astype(BF16)
    td_bf = trend_direction.astype(BF16)

    in_maps = []
    for c in range(N_CORES):
        sl = slice(c * n_per_core, (c + 1) * n_per_core)
        tg = tgt_bf[sl].reshape(p, f)
        pc2 = pc_bf[sl].reshape(p, f)
        td2 = td_bf[sl].reshape(p, f)
        blocks = []
        for off, tk in zip(offs, sizes):
            blocks.append(tg[:, off : off + tk])
            blocks.append(pc2[:, off : off + tk])
            blocks.append(td2[:, off : off + tk])
        auxv = np.concatenate(blocks, axis=1)  # [p, 3*f]
        in_maps.append(
            {
                "pred": np.ascontiguousarray(pred_bf[sl]).reshape(p, f, C),
                "aux": np.ascontiguousarray(auxv),
            }
        )
    return in_maps


def combine(results):
    """Host-side reduction of per-core partial sums -> final scalar loss."""
    s_ce = s_w = s_ap = s_al = 0.0
    for r in results:
        s_ce += float(r["ce_out"].astype(np.float64).sum())
        s_w += float(r["w_out"].astype(np.float64).sum())
        s_ap += float(r["ap_out"].astype(np.float64).sum())
        s_al += float(r["al_out"].astype(np.float64).sum())

    mean_ap = s_ap / B
    weighted_ce_mean = (s_w / B) / (mean_ap + EPS)
    ce_mean = s_ce / B
    trend_mean = -0.1 * s_al / B
    loss = (
        DIRECTIONAL_WEIGHT * weighted_ce_mean
        + MAGNITUDE_WEIGHT * ce_mean
        + TREND_WEIGHT * trend_mean
    )
    return np.float32(loss)


def kernel(predictions, targets, price_changes, trend_direction):
    nc = _get_nc()
    in_maps = make_in_maps(predictions, targets, price_changes, trend_direction)
    last_err = None
    for _attempt in range(3):
        try:
            res = run_bass_kernel_spmd(nc, in_maps, core_ids=list(range(N_CORES)))
            return combine(res.results)
        except Exception as e:  # rare transient NRT_EXEC_UNIT_UNRECOVERABLE
            last_err = e
    raise last_err


# revision 32
# speedup vs baseline: 1.0397x; 1.0087x over previous
"""Trainium2 Bass kernel for CustomTradingLoss.

Computes, over B=8388608 samples with C=3 classes:
    ce      = logsumexp(pred) - pred[target]          (per sample)
    loss    = 0.85 * mean(ce * |pc|) / (mean(|pc|) + 1e-8)
            + 0.15 * mean(ce)
            + 0.1  * mean(where(aligned, -0.1, 0))
    aligned = (td > 0 & t == 2) | (td < 0 & t == 0)  == ((t-1)*td > 0)

Pure data parallel across 8 NeuronCores: core c gets samples
[c*B/8, (c+1)*B/8), laid out [128 partitions x 8192 free]. Each core
emits partial sums; the host reduces them in f64 and applies the final
formula (the three means only need global sums, so no collectives).

The on-device datapath runs in bf16 (inputs are cast host-side):
  - halves HBM traffic (the kernel is memory-bound at f32)
  - unlocks DVE 2x/4x perf modes (fp32 tensor_tensor is capped at 1x)
Targets {0,1,2} and all signs are exact in bf16; the quantization noise
on ce is ~0.4% zero-mean per sample and averages out over 8.4M samples
(measured end-to-end rel err ~1e-4 vs the f32 reference).

Engine placement notes (hardware-measured):
  - GpSimd must stay IDLE: any Pool op holds the DVE-shared SBUF port
    for its whole (slow) duration, stalling every 2-input DVE op.
  - tensor_tensor_reduce crashes this HW; sums of ce/w/al instead go
    through the otherwise-idle PE as ones-vector matmuls accumulating
    in PSUM (f32), which costs the DVE nothing.
  - bass's activation-table chooser is first-match; without forcing a
    single combined exp+ln set it reloads tables every tile.
"""

import os
import sys

import numpy as np

for _p in ("/opt/trn_rl_repo", "/opt/trn_rl_repo/concourse"):
    if os.path.isdir(_p) and _p not in sys.path:
        sys.path.insert(0, _p)

import ml_dtypes

import concourse.bacc as bacc
import concourse.mybir as mybir
import concourse.tile as tile
from concourse.bass_utils import run_bass_kernel_spmd

B = 8388608
C = 3
N_CORES = 8
N_PER_CORE = B // N_CORES  # 1048576
P = 128
F = N_PER_CORE // P  # 8192 free elements per partition
T = 2048  # tile free size

DIRECTIONAL_WEIGHT = 0.85
MAGNITUDE_WEIGHT = 0.15
TREND_WEIGHT = 0.1
EPS = 1e-8

f32 = mybir.dt.float32
bf16 = mybir.dt.bfloat16
u16 = mybir.dt.uint16
AF = mybir.ActivationFunctionType
OP = mybir.AluOpType
BF16 = ml_dtypes.bfloat16


def _force_single_act_table():
    """Make both bass and walrus use natural_log_exp_and_others (covers
    exp, ln, abs, copy, relu...) as the only activation table set, as set
    id 0 on both sides. Without this, bass's first-match set chooser
    alternates exp/ln table loads every tile (~1.3us each + a bubble).

    Two halves that must stay consistent:
      - bass picks set ids from hw_specs.get_activation_tables -> patch
        bacc's binding to a single-entry dict (id 0 = the combined set)
      - walrus validates/loads ids against act_info.json -> point
        BASS_ACT_ROOT_JSON_PATH at a filtered copy with just that set
    """
    import concourse.hw_specs as hw_specs

    name = "natural_log_exp_and_others"
    tables = hw_specs.get_activation_tables("gen3")
    if name in tables:
        bacc.get_activation_tables = lambda arch: {name: tables[name]}

    if os.environ.get("BASS_ACT_ROOT_JSON_PATH"):
        return
    import glob
    import json
    import shutil
    import tempfile

    import neuronxcc

    hits = glob.glob(
        os.path.join(os.path.dirname(neuronxcc.__file__), "pwp", "*", "act_info.json")
    )
    if not hits:
        return
    src = hits[0]
    d = json.load(open(src))
    keep = [s for s in d.get("act_func_sets", []) if s.get("name") == name]
    if not keep:
        return
    tmpdir = tempfile.mkdtemp(prefix="act_single_")
    for fn in os.listdir(os.path.dirname(src)):
        srcf = os.path.join(os.path.dirname(src), fn)
        if os.path.isfile(srcf) and fn != "act_info.json":
            try:
                os.symlink(srcf, os.path.join(tmpdir, fn))
            except OSError:
                shutil.copy(srcf, os.path.join(tmpdir, fn))
    d["act_func_sets"] = keep
    with open(os.path.join(tmpdir, "act_info.json"), "w") as f:
        json.dump(d, f)
    os.environ["BASS_ACT_ROOT_JSON_PATH"] = os.path.join(tmpdir, "act_info.json")


def _tile_sizes(f, t):
    """Short leading tiles (cheap pipeline fill), then full tiles."""
    sizes = [t // 4, t // 4, t // 2] + [t] * (f // t - 1)
    assert sum(sizes) == f
    return sizes


def build(p=P, f=F, t=T, inp_bufs=3, work_bufs=2):
    """Build + compile the per-core program. Same program on all 8 cores.

    Inputs (bf16, packed host-side):
      pred [p, f, 3]   per-sample class logits (interleaved)
      aux  [p, 3*f]    per tile k: [targets | price_changes | trend] blocks
                       of that tile's size, concatenated in tile order
    Outputs (f32): ce/w/al/ap [1, nsum] column partial sums (PE/PSUM).
    """
    _force_single_act_table()
    sizes = _tile_sizes(f, t)
    offs = [sum(sizes[:i]) for i in range(len(sizes))]
    last = len(sizes) - 1
    chunk = min(512, min(sizes))  # PE matmul column width; divides every size
    assert all(s % chunk == 0 for s in sizes)
    nsum = chunk

    nc = bacc.Bacc(
        "TRN2", target_bir_lowering=False, debug=False, num_devices=N_CORES
    )

    pred = nc.dram_tensor("pred", [p, f, C], bf16, kind="ExternalInput").ap()
    aux = nc.dram_tensor("aux", [p, 3 * f], bf16, kind="ExternalInput").ap()
    ce_out = nc.dram_tensor("ce_out", [1, nsum], f32, kind="ExternalOutput").ap()
    w_out = nc.dram_tensor("w_out", [1, nsum], f32, kind="ExternalOutput").ap()
    al_out = nc.dram_tensor("al_out", [1, nsum], f32, kind="ExternalOutput").ap()
    ap_out = nc.dram_tensor("ap_out", [1, nsum], f32, kind="ExternalOutput").ap()

    with tile.TileContext(nc) as tc:
        with (
            tc.tile_pool(name="inp", bufs=inp_bufs) as inp,
            tc.tile_pool(name="work", bufs=work_bufs) as work,
            tc.tile_pool(name="acc", bufs=1) as acc,
            tc.tile_pool(name="psum", bufs=1, space="PSUM") as psum,
        ):
            ones = acc.tile([p, 1], bf16, tag="ones")
            nc.vector.memset(ones[:], 1.0)
            ps_ce = psum.tile([1, nsum], f32, tag="ps_ce")
            ps_w = psum.tile([1, nsum], f32, tag="ps_w")
            ps_al = psum.tile([1, nsum], f32, tag="ps_al")
            ps_ap = psum.tile([1, nsum], f32, tag="ps_ap")

            def pe_sum(ps, x, k, tk):
                for j in range(tk // chunk):
                    nc.tensor.matmul(
                        ps[:],
                        ones[:],
                        x[:, j * chunk : (j + 1) * chunk],
                        start=(k == 0 and j == 0),
                        stop=(k == last and j == tk // chunk - 1),
                    )

            for k, (off, tk) in enumerate(zip(offs, sizes)):
                # ax first: the mask/trend chain only needs ax, so it can
                # start while the (3x larger) pt transfer still streams
                ax = inp.tile([p, 3, tk], bf16, tag="ax")
                axd = aux[:, 3 * off : 3 * (off + tk)].rearrange(
                    "p (c t) -> p c t", c=3
                )
                nc.sync.dma_start(out=ax[:], in_=axd[:])
                pt = inp.tile([p, tk, C], bf16, tag="pt")
                nc.sync.dma_start(out=pt[:], in_=pred[:, off : off + tk, :])
                tt = ax[:, 0, :]
                pct = ax[:, 1, :]
                tdt = ax[:, 2, :]

                # e_j = exp(pred_j), deinterleaved to unit-stride bf16 (ACT)
                e0 = work.tile([p, tk], bf16, tag="e0")
                e1 = work.tile([p, tk], bf16, tag="e1")
                e2 = work.tile([p, tk], bf16, tag="e2")
                nc.scalar.activation(e0[:], pt[:, :, 0], AF.Exp)
                nc.scalar.activation(e1[:], pt[:, :, 1], AF.Exp)
                nc.scalar.activation(e2[:], pt[:, :, 2], AF.Exp)

                # s = e0 + e1 + e2 (DVE bf16 2x); lse = ln(s) (ACT)
                s01 = work.tile([p, tk], bf16, tag="s01")
                nc.vector.tensor_add(s01[:], e0[:], e1[:])
                s = work.tile([p, tk], bf16, tag="s")
                nc.vector.tensor_add(s[:], s01[:], e2[:])
                lse = work.tile([p, tk], bf16, tag="lse")
                nc.scalar.activation(lse[:], s[:], AF.Ln)

                # masks for target selection: bf16 is_equal runs at DVE 4x;
                # the 1.0/0.0 bf16 pattern bitcasts to a valid uint16
                # predicate for copy_predicated. (GpSimd must stay idle --
                # see module docstring)
                m0 = work.tile([p, tk], bf16, tag="m0")
                nc.vector.tensor_scalar(
                    out=m0[:], in0=tt, scalar1=0.0, scalar2=None, op0=OP.is_equal
                )
                m2 = work.tile([p, tk], bf16, tag="m2")
                nc.vector.tensor_scalar(
                    out=m2[:], in0=tt, scalar1=2.0, scalar2=None, op0=OP.is_equal
                )

                # e1 <- e[target] via predicated overwrites (DVE), then ln
                nc.vector.copy_predicated(
                    out=e1[:], mask=m2[:].bitcast(u16), data=e2[:]
                )
                nc.vector.copy_predicated(
                    out=e1[:], mask=m0[:].bitcast(u16), data=e0[:]
                )
                lsel = work.tile([p, tk], bf16, tag="lsel")
                nc.scalar.activation(lsel[:], e1[:], AF.Ln)

                # ap = |pc| by clearing the bf16 sign bit (DVE 4x int op;
                # cheaper than an ACT Abs pass). Sum goes through PE.
                apb = work.tile([p, tk], u16, tag="apb")
                nc.vector.tensor_scalar(
                    out=apb[:],
                    in0=pct.bitcast(u16),
                    scalar1=0x7FFF,
                    scalar2=None,
                    op0=OP.bitwise_and,
                )
                apt = apb[:].bitcast(bf16)

                # ce = lse - lsel; w = ce * ap  (DVE 2x TT; sums on PE)
                ce = work.tile([p, tk], bf16, tag="ce")
                nc.vector.tensor_sub(ce[:], lse[:], lsel[:])
                w = work.tile([p, tk], bf16, tag="w")
                nc.vector.tensor_mul(w[:], ce[:], apt)

                # aligned = ((t-1)*td > 0)  (DVE; sum on PE)
                u = work.tile([p, tk], bf16, tag="u")
                nc.vector.tensor_scalar(
                    out=u[:], in0=tt, scalar1=1.0, scalar2=None, op0=OP.subtract
                )
                q = work.tile([p, tk], bf16, tag="q")
                nc.vector.tensor_mul(q[:], u[:], tdt)
                al = work.tile([p, tk], bf16, tag="al")
                nc.vector.tensor_scalar(
                    out=al[:], in0=q[:], scalar1=0.0, scalar2=None, op0=OP.is_gt
                )

                pe_sum(ps_ce, ce[:], k, tk)
                pe_sum(ps_w, w[:], k, tk)
                pe_sum(ps_al, al[:], k, tk)
                pe_sum(ps_ap, apt, k, tk)

            sums = acc.tile([1, 4, nsum], f32, tag="sums")
            nc.vector.tensor_copy(out=sums[:, 0, :], in_=ps_ce[:])
            nc.vector.tensor_copy(out=sums[:, 1, :], in_=ps_w[:])
            nc.vector.tensor_copy(out=sums[:, 2, :], in_=ps_al[:])
            nc.vector.tensor_copy(out=sums[:, 3, :], in_=ps_ap[:])
            nc.sync.dma_start(out=ce_out[:], in_=sums[:, 0, :])
            nc.sync.dma_start(out=w_out[:], in_=sums[:, 1, :])
            nc.sync.dma_start(out=al_out[:], in_=sums[:, 2, :])
            nc.sync.dma_start(out=ap_out[:], in_=sums[:, 3, :])

    nc.compile()
    return nc


_NC = None


def _get_nc():
    global _NC
    if _NC is None:
        _NC = build()
    return _NC


def make_in_maps(predictions, targets, price_changes, trend_direction, p=P, t=T):
    """Shard across cores and pack into the kernel's bf16 input layout."""
    predictions = np.asarray(predictions)
    targets = np.asarray(targets)
    price_changes = np.asarray(price_changes)
    trend_direction = np.asarray(trend_direction)

    n = predictions.shape[0]
    n_per_core = n // N_CORES
    f = n_per_core // p
    sizes = _tile_sizes(f, t)
    offs = [sum(sizes[:i]) for i in range(len(sizes))]

    pred_bf = predictions.astype(BF16)
    tgt_bf = targets.astype(BF16)
    pc_bf = price_changes.astype(BF16)
    td_bf = trend_direction.astype(BF16)

    in_maps = []
    for c in range(N_CORES):
        sl = slice(c * n_per_core, (c + 1) * n_per_core)
        tg = tgt_bf[sl].reshape(p, f)
        pc2 = pc_bf[sl].reshape(p, f)
        td2 = td_bf[sl].reshape(p, f)
        blocks = []
        for off, tk in zip(offs, sizes):
            blocks.append(tg[:, off : off + tk])
            blocks.append(pc2[:, off : off + tk])
            blocks.append(td2[:, off : off + tk])
        auxv = np.concatenate(blocks, axis=1)  # [p, 3*f]
        in_maps.append(
            {
                "pred": np.ascontiguousarray(pred_bf[sl]).reshape(p, f, C),
                "aux": np.ascontiguousarray(auxv),
            }
        )
    return in_maps


def combine(results):
    """Host-side reduction of per-core partial sums -> final scalar loss."""
    s_ce = s_w = s_ap = s_al = 0.0
    for r in results:
        s_ce += float(r["ce_out"].astype(np.float64).sum())
        s_w += float(r["w_out"].astype(np.float64).sum())
        s_ap += float(r["ap_out"].astype(np.float64).sum())
        s_al += float(r["al_out"].astype(np.float64).sum())

    mean_ap = s_ap / B
    weighted_ce_mean = (s_w / B) / (mean_ap + EPS)
    ce_mean = s_ce / B
    trend_mean = -0.1 * s_al / B
    loss = (
        DIRECTIONAL_WEIGHT * weighted_ce_mean
        + MAGNITUDE_WEIGHT * ce_mean
        + TREND_WEIGHT * trend_mean
    )
    return np.float32(loss)


def kernel(predictions, targets, price_changes, trend_direction):
    nc = _get_nc()
    in_maps = make_in_maps(predictions, targets, price_changes, trend_direction)
    last_err = None
    for _attempt in range(3):
        try:
            res = run_bass_kernel_spmd(nc, in_maps, core_ids=list(range(N_CORES)))
            return combine(res.results)
        except Exception as e:  # rare transient NRT_EXEC_UNIT_UNRECOVERABLE
            last_err = e
    raise last_err
